# revision 28
# baseline (speedup 1.0000x reference)
"""Block-sparse linear y = x @ W^T + bias on 8 Trainium2 NeuronCores.

W [4096, 4096] has K=1024 dense 64x64 tiles at (row_idx[k], col_idx[k]) on a
64x64 block grid. Data-parallel over tokens: each core gets 512 rows of x and
all blocks, computing yT_local [4096, 512] = W @ x_local^T + bias.

Per-core device layout:
  - xT in SBUF (bf16) as one [128, nslot*512] image: input column blocks
    are paired into "slots" (first member on partitions 0-63, second on
    64-127), so a block's PE quadrant parity is its pair position. The
    pairing and orientation are optimized on the host to balance the four
    (parity, half) quadrant populations per chunk (per-chunk PE time is
    the max quadrant count); slots are ordered by first use across the
    chunk schedule for just-in-time prefetch.
  - all block weights SBUF-resident as one [128, wtot*64] bf16 image of
    B_k^T tiles (parity-0 on partitions 0-63, parity-1 on 64-127).
  - output block-rows are processed in 32 PSUM generations ("chunks") of 2
    rows (one per 64-partition half), rows paired by similar block count;
    parity-p contributions accumulate in psum bank 2k+p (a PSUM
    accumulation group must keep one tile_position). Banks rotate 4-deep.
  - drain: ACT adds bias while copying the even bank to SBUF (half-column
    pieces for a short latency chain), DVE adds the odd bank and writes
    bf16; batched [128, 4*512] bf16 stores go out on the scalar HWDGE
    queue. The host converts back to f32.

All inbound DMAs are EDF-interleaved across the sync and scalar HWDGE
queues (each drains FIFO, sharing HBM at packet granularity) so compute
prerequisites land just in time and the PE, once started, never starves
(the PE p-state ramp 0.65->1.2->2.4 GHz resets on idle, so continuous busy
= full clock; zero-tile warmup matmuls bridge the initial load phase).

The schedule is specialized on the host from row_idx/col_idx (duplicate
(r,c) blocks are pre-summed); all matmul FLOPs run on the PE.

This toolchain's walrus rejects >1 semaphore wait per instruction, so after
Tile scheduling we split excess waits onto same-engine NoOps.
"""

import numpy as np
import ml_dtypes

N_TOK, IN_F, OUT_F, BS, NCORES = 4096, 4096, 4096, 64, 8
NL = N_TOK // NCORES            # tokens per core (512)
GRID = OUT_F // BS              # 64 block-rows / block-cols
NCHUNK = 32                     # psum generations, 2 block-rows each
XB = 4                          # slots per x DMA batch
OB = 4                          # chunks per output DMA batch
NWARM = 40                      # zero-tile warmup matmuls: they both keep
                                # the PE p-state ramp warm through the load
                                # phase and gate real compute until the
                                # inbound stream is far enough ahead to
                                # sustain it stall-free
SCAL_CAP = 3 * 1024 * 1024      # inbound bytes allowed on the scalar queue

_CACHE = {}


def _plan(row_idx, col_idx):
    from collections import OrderedDict

    K = int(row_idx.shape[0])
    cells = OrderedDict()
    for k in range(K):
        cells.setdefault((int(row_idx[k]), int(col_idx[k])), []).append(k)
    rows = {r: [] for r in range(GRID)}
    for (r, c) in cells:
        rows[r].append(c)
    counts = {r: len(rows[r]) for r in range(GRID)}

    # chunk 0: the row pair with the smallest column-set union, so the first
    # chunk's x and weight prerequisites are minimal
    best = None
    for r1 in range(GRID):
        for r2 in range(r1 + 1, GRID):
            u = len(set(rows[r1]) | set(rows[r2]))
            if best is None or u < best[0]:
                best = (u, r1, r2)
    first_pair = [best[1], best[2]]

    # remaining chunks pair rows of SIMILAR block count: per-chunk PE time
    # is ceil(max(n_top, n_bot)/2) quadrant-serial matmuls, so similar
    # counts minimize the total
    rest = sorted((r for r in range(GRID) if r not in first_pair),
                  key=lambda r: -counts[r])
    chunk_rows = [[rest[2 * i], rest[2 * i + 1]]
                  for i in range(len(rest) // 2)]

    # order chunks greedily by fewest new columns, compressing the early
    # x-prefetch demand so compute can start while most of x is in flight
    cur = set(c for r in first_pair for c in rows[r])
    rem = list(chunk_rows)
    ordered = [first_pair]
    while rem:
        nxt = min(rem, key=lambda rs: len({c for r in rs for c in rows[r]}
                                          - cur))
        rem.remove(nxt)
        ordered.append(nxt)
        cur |= {c for r in nxt for c in rows[r]}
    # tail chunks big-to-small: the final chunk's matmuls then finish just
    # after the rest, shortening the last drain->store chain
    tail = sorted(ordered[-6:],
                  key=lambda rs: -sum(counts[r] for r in rs))
    chunk_rows = ordered[:-6] + tail

    # half assignment per row (which 64-partition output half of its chunk)
    half_of = {}
    for rs in chunk_rows:
        a, b = sorted(rs)
        half_of[a], half_of[b] = 0, 1
    chunk_of = {r: i for i, rs in enumerate(chunk_rows) for r in rs}
    colrows = {c: [r for r in range(GRID) if c in rows[r]]
               for c in range(GRID)}

    # slot pairing: two columns share a slot (one per partition half, so
    # its parity is pair position). Pair in first-use order for prefetch
    # locality, choosing partners/orientation to balance per-row parity
    # counts (per-chunk PE time is the max quadrant population), then
    # hill-climb orientation flips and windowed partner swaps.
    D = {r: 0 for r in range(GRID)}     # row parity deficit (p0 - p1)

    def orient_score(ca, cb):
        s = 0
        for r in colrows[ca]:
            s += abs(D[r] + 1) - abs(D[r])
        for r in colrows[cb]:
            s += abs(D[r] - 1) - abs(D[r])
        return s

    def commit(ca, cb):
        for r in colrows[ca]:
            D[r] += 1
        for r in colrows[cb]:
            D[r] -= 1

    def best_partner(ca, pend):
        bestp = None
        for j, cb in enumerate(pend):
            for o in (0, 1):
                s = orient_score(ca, cb) if o == 0 else orient_score(cb, ca)
                if bestp is None or s < bestp[0]:
                    bestp = (s, j, o)
        return bestp

    seen, pend, pairs = set(), [], []
    for rs in chunk_rows:
        for c in sorted({c for r in rs for c in rows[r]}):
            if c in seen:
                continue
            seen.add(c)
            pend.append(c)
            if len(pend) >= 6:
                ca = pend.pop(0)
                s, j, o = best_partner(ca, pend)
                cb = pend.pop(j)
                pr = (ca, cb) if o == 0 else (cb, ca)
                pairs.append(list(pr))
                commit(*pr)
    while len(pend) >= 2:
        ca = pend.pop(0)
        s, j, o = best_partner(ca, pend)
        cb = pend.pop(j)
        pr = (ca, cb) if o == 0 else (cb, ca)
        pairs.append(list(pr))
        commit(*pr)
    if pend:
        pairs.append([pend[0], pend[0]])

    def sigma_max(prs):
        par = {}
        for (a, b) in prs:
            par[a] = 0
            if b != a:
                par[b] = 1
        tot = 0
        for rs in chunk_rows:
            q = {(p, h): 0 for p in (0, 1) for h in (0, 1)}
            for r in rs:
                for c in rows[r]:
                    q[(par[c], half_of[r])] += 1
            tot += max(q.values())
        return tot

    cur = sigma_max(pairs)
    for _ in range(40):
        improved = False
        for i in range(len(pairs)):
            pairs[i] = [pairs[i][1], pairs[i][0]]
            s = sigma_max(pairs)
            if s < cur:
                cur, improved = s, True
            else:
                pairs[i] = [pairs[i][1], pairs[i][0]]
        for i in range(len(pairs)):
            for j in range(i + 1, min(i + 7, len(pairs))):
                done = False
                for si, sj in ((0, 0), (0, 1), (1, 0), (1, 1)):
                    a, b = pairs[i][:], pairs[j][:]
                    a[si], b[sj] = b[sj], a[si]
                    old_i, old_j = pairs[i], pairs[j]
                    pairs[i], pairs[j] = a, b
                    s = sigma_max(pairs)
                    if s < cur:
                        cur, improved, done = s, True, True
                        break
                    pairs[i], pairs[j] = old_i, old_j
                if done:
                    break
        if not improved:
            break

    slot, par = {}, {}
    for s, (a, b) in enumerate(pairs):
        slot[a], par[a] = s, 0
        if b != a:
            slot[b], par[b] = s, 1
    nslot = len(pairs)

    chunks = []
    xdead = {}                  # slot -> first chunk index that reads it
    for ci, rs in enumerate(chunk_rows):
        rs = sorted(rs, key=lambda r: -counts[r])
        regions = {(0, 0): rs[0], (0, 1): rs[1]}
        if rs[0] > rs[1]:
            regions = {(0, 0): rs[1], (0, 1): rs[0]}
        rloc = {r: kh for kh, r in regions.items()}

        queues = {(p, h): [] for p in (0, 1) for h in (0, 1)}
        for r in rs:
            k, h = rloc[r]
            for c in sorted(rows[r], key=lambda c: slot[c]):
                queues[(par[c], h)].append((r, c))
                xdead.setdefault(slot[c], ci)

        cycle = [(0, 0), (1, 1), (1, 0), (0, 1)]
        ent = []
        qi = {kq: 0 for kq in queues}
        total = sum(len(q) for q in queues.values())
        while len(ent) < total:
            prog = False
            for kq in cycle:
                q = queues[kq]
                if qi[kq] < len(q):
                    ent.append((q[qi[kq]], kq[0]))
                    qi[kq] += 1
                    prog = True
            assert prog
        # start/stop per accumulator = (region, parity)
        first, last = {}, {}
        for i, ((r, c), p) in enumerate(ent):
            first.setdefault((r, p), i)
            last[(r, p)] = i
        entries = []
        for i, ((r, c), p) in enumerate(ent):
            k, h = rloc[r]
            entries.append(dict(r=r, c=c, p=p, slot=slot[c], half=h,
                               start=(first[(r, p)] == i),
                               stop=(last[(r, p)] == i)))
        # accumulators with no blocks -> memset
        empty = []
        for (k, h), r in regions.items():
            for p in (0, 1):
                if not any(par[c] == p for c in rows[r]):
                    empty.append((k, h, p))
        chunks.append(dict(rows=rs, regions=regions, entries=entries,
                           empty=empty))

    return dict(cells=cells, chunks=chunks, pairs=pairs, nslot=nslot,
                xdead=xdead)


def _build_images(plan, blocks, bias):
    cells = plan["cells"]
    summed = {}
    for key, ks in cells.items():
        if len(ks) == 1:
            summed[key] = blocks[ks[0]]
        else:
            acc = blocks[ks[0]].astype(np.float32).copy()
            for k in ks[1:]:
                acc += blocks[k]
            summed[key] = acc

    # one interleaved image: per chunk, parity-0 blocks on partitions 0-63
    # and parity-1 on 64-127, column-aligned and zero padded to the wider half
    widths = []
    for ch in plan["chunks"]:
        n_e = sum(1 for e in ch["entries"] if e["p"] == 0)
        n_o = len(ch["entries"]) - n_e
        widths.append(max(n_e, n_o, 1))
    wtot = sum(widths)
    img = np.zeros((128, wtot * BS), np.float32)
    seg = []  # per chunk: (col offset in blocks, width in blocks)
    off = 0
    for wch, ch in zip(widths, plan["chunks"]):
        ie = io = 0
        for e in ch["entries"]:
            B = summed[(e["r"], e["c"])]
            if e["p"] == 0:
                img[0:64, (off + ie) * BS:(off + ie + 1) * BS] = B.T
                e["loc"] = ie
                ie += 1
            else:
                img[64:128, (off + io) * BS:(off + io + 1) * BS] = B.T
                e["loc"] = io
                io += 1
        seg.append((off, wch))
        off += wch

    bias_img = np.zeros((128, NCHUNK), np.float32)
    for ci, ch in enumerate(plan["chunks"]):
        for (k, h), r in ch["regions"].items():
            bias_img[h * 64:(h + 1) * 64, ci] = bias[r * BS:(r + 1) * BS]

    return img.astype(ml_dtypes.bfloat16), bias_img, seg


def _split_excess_waits(nc, mybir, limit=1):
    n = 0
    for fn in nc.m.functions:
        for bb in fn.blocks:
            out = []
            for inst in bb.instructions:
                si = inst.sync_info
                if si is not None and si.on_wait and len(si.on_wait) > limit:
                    waits = list(si.on_wait)
                    ups = list(si.on_update)
                    for j, w in enumerate(waits[:-limit]):
                        nop = mybir.InstNoOp(name=f"{inst.name}-ws{j}", ins=[], outs=[])
                        nop.engine = inst.engine
                        nop.sync_info = mybir.SyncInfo(on_wait=[w], on_update=[])
                        out.append(nop)
                        n += 1
                    inst.sync_info = mybir.SyncInfo(on_wait=waits[-limit:], on_update=ups)
                out.append(inst)
            bb.instructions = out
    return n


def _thin_engine_sem_updates(nc, mybir, engines=("EngineType.PE",)):
    """Drop per-instruction +1 sem increments that no wait ever observes.

    Tile gives every engine instruction a `then_inc(engine_sem)`; on the PE a
    serialized EVT_SEM write costs ~26 ns per matmul. Only ticks some other
    instruction actually waits on are needed, so keep increments just before
    each waited tick and renumber all waits by rank.
    """
    insts = []
    for fn in nc.m.functions:
        for bb in fn.blocks:
            insts.extend(bb.instructions)

    from collections import defaultdict
    upd_insts = defaultdict(list)   # sem id -> [instruction, ...] program order
    upd_ok = defaultdict(lambda: True)
    upd_engine = {}
    waited = defaultdict(set)       # sem id -> waited values
    wait_ok = defaultdict(lambda: True)
    for inst in insts:
        si = inst.sync_info
        if si is None:
            continue
        for u in si.on_update:
            if u.sync_type != "semaphore":
                continue
            if u.update_mode != "sem-inc" or u.update_value != 1:
                upd_ok[u.id] = False
            e = str(inst.engine)
            if u.id in upd_engine and upd_engine[u.id] != e:
                upd_ok[u.id] = False
            upd_engine[u.id] = e
            upd_insts[u.id].append(inst)
        for w in si.on_wait:
            if w.sync_type != "semaphore":
                continue
            if w.wait_mode != "sem-ge-imm" or w.wait_reg is not None:
                wait_ok[w.id] = False
            waited[w.id].add(w.wait_value)

    victims = [s for s, il in upd_insts.items()
               if upd_ok[s] and wait_ok[s] and upd_engine.get(s) in engines
               and len(il) > 8]
    for s in victims:
        il = upd_insts[s]
        W = sorted(v for v in waited.get(s, set()) if 1 <= v <= len(il))
        keep_ticks = set(W)
        rank = {v: i + 1 for i, v in enumerate(W)}
        # always keep the final tick so the kernel tail drain can await it
        if len(il) not in keep_ticks:
            keep_ticks.add(len(il))
            rank[len(il)] = len(W) + 1
        for t, inst in enumerate(il, start=1):
            si = inst.sync_info
            ups = [u for u in si.on_update
                   if not (u.sync_type == "semaphore" and u.id == s)]
            if t in keep_ticks:
                ups.append(mybir.SyncUpdate(
                    sync_type="semaphore", id=s, ant_name=f"thin{s}",
                    update_mode="sem-inc", update_value=1, update_reg=None))
            inst.sync_info = mybir.SyncInfo(on_wait=list(si.on_wait),
                                            on_update=ups)
        # renumber waits on this sem everywhere
        for inst in insts:
            si = inst.sync_info
            if si is None or not si.on_wait:
                continue
            changed = False
            ws = []
            for w in si.on_wait:
                if w.sync_type == "semaphore" and w.id == s:
                    nv = rank.get(w.wait_value)
                    if nv is None:
                        nv = sum(1 for v in rank if v <= w.wait_value)
                    ws.append(mybir.SyncWait(
                        sync_type="semaphore", id=s, ant_name=f"thin{s}",
                        wait_mode="sem-ge-imm", wait_value=nv, wait_reg=None))
                    changed = True
                else:
                    ws.append(w)
            if changed:
                inst.sync_info = mybir.SyncInfo(on_wait=ws,
                                                on_update=list(si.on_update))
    return victims


def _build_bass(plan, wimg, seg, nslot):
    import concourse.bass as bass
    import concourse.mybir as mybir
    import concourse.tile as tile

    F32 = mybir.dt.float32
    BF16 = mybir.dt.bfloat16

    nc = bass.Bass()
    xTd = nc.declare_dram_parameter("xT", [128, nslot * NL], BF16, isOutput=False)
    imd = nc.declare_dram_parameter("img", [128, wimg * BS], BF16, isOutput=False)
    bd = nc.declare_dram_parameter("bias_img", [128, NCHUNK], F32, isOutput=False)
    yTd = nc.declare_dram_parameter("yT", [128, NCHUNK * NL], BF16, isOutput=True)

    nxb = (nslot + XB - 1) // XB
    # weight batches: pairs for the first chunks, then fours
    wbounds = [0, 2, 4, 6] + list(range(8, NCHUNK, 4)) + [NCHUNK]
    # output batches: fours, with a small tail for a short drain->store chain
    obounds = list(range(0, NCHUNK - 4, OB)) + [NCHUNK - 4, NCHUNK - 2,
                                                NCHUNK - 1, NCHUNK]
    ob_of = {}
    for b in range(len(obounds) - 1):
        for ci in range(obounds[b], obounds[b + 1]):
            ob_of[ci] = b

    def wslice(b):
        c0, c1 = wbounds[b], wbounds[b + 1] - 1
        return seg[c0][0] * BS, (seg[c1][0] + seg[c1][1]) * BS

    def xslice(b):
        return b * XB * NL, min(nslot, (b + 1) * XB) * NL

    # EDF merge of inbound loads: deadline = first chunk that reads the
    # batch. Weights go on the sync queue (scalar later carries output
    # stores); x batches alternate between the queues up to SCAL_CAP.
    items = []
    for b in range(nxb):
        d = min(plan["xdead"].get(s, NCHUNK)
                for s in range(b * XB, min(nslot, (b + 1) * XB)))
        lo, hi = xslice(b)
        items.append((d, 1, "x", b, (hi - lo) * 256))
    for b in range(len(wbounds) - 1):
        lo, hi = wslice(b)
        items.append((wbounds[b], 0, "w", b, (hi - lo) * 256))
    items.sort()
    # balance the EDF stream across three inbound queues by bytes: weights
    # stay on sync, x also rides scalar (clears before output stores need
    # it) and gpsimd (SWDGE) during the early crunch
    sync_q, scal_q, gp_q = [], [], []
    qb = {"sync": 0.0, "scal": 0.0, "gp": 0.0}
    for d, _, kind, b, sz in items:
        cands = ["sync"]
        if kind == "x":
            if qb["scal"] + sz <= SCAL_CAP:
                cands.append("scal")
            if d <= 10 and qb["gp"] + sz <= 2 * 1024 * 1024:
                cands.append("gp")
        q = min(cands, key=lambda n: qb[n])
        qb[q] += sz
        {"sync": sync_q, "scal": scal_q, "gp": gp_q}[q].append((kind, b))
    scal_q.insert(min(2, len(scal_q)), ("bias", 0))

    with tile.TileContext(nc) as tc:
        with (
            tc.tile_pool(name="big", bufs=1) as big_pool,
            tc.tile_pool(name="cst", bufs=1) as cst_pool,
            tc.tile_pool(name="stp", bufs=3) as st_pool,
            tc.tile_pool(name="tmp", bufs=4) as tmp_pool,
            tc.tile_pool(name="ps", bufs=1, space="PSUM") as ps_pool,
        ):
            zblk = cst_pool.tile([128, BS], BF16, tag="zblk")
            nc.vector.memset(zblk[:], 0.0)
            wscr = cst_pool.tile([128, NL], BF16, tag="wscr")
            nc.vector.memset(wscr[:], 0.0)
            bias_t = cst_pool.tile([128, NCHUNK], F32, tag="bias")

            xt_t = big_pool.tile([128, nslot * NL], BF16, tag="xt")
            wt_t = big_pool.tile([128, wimg * BS], BF16, tag="wt")

            for q, eng in ((sync_q, nc.sync), (scal_q, nc.scalar),
                           (gp_q, nc.gpsimd)):
                for kind, b in q:
                    if kind == "x":
                        lo, hi = xslice(b)
                        eng.dma_start(out=xt_t[:, lo:hi], in_=xTd[:, lo:hi])
                    elif kind == "w":
                        lo, hi = wslice(b)
                        eng.dma_start(out=wt_t[:, lo:hi], in_=imd[:, lo:hi])
                    else:
                        eng.dma_start(out=bias_t[:], in_=bd[:])

            # warmup matmuls on zero tiles: keep the PE busy through the
            # load phase so the p-state/HAM ramp is done when real work lands
            ps_warm = ps_pool.tile([128, NL], F32, tag="bank6", name="warm")
            for i in range(NWARM):
                h = i % 2
                nc.tensor.matmul(ps_warm[h * 64:(h + 1) * 64, :],
                                 zblk[0:64, :], wscr[0:64, :],
                                 start=True, stop=True,
                                 tile_position=(0, h * 64))

            for ci, ch in enumerate(plan["chunks"]):
                coff = seg[ci][0]
                boff = 2 * (ci % 4)
                ps_tiles = [ps_pool.tile([128, NL], F32, tag=f"bank{boff+b}",
                                         name=f"ps{ci}_{b}")
                            for b in range(2)]
                for (k, h, p) in ch["empty"]:
                    nc.tensor.matmul(
                        ps_tiles[p][h * 64:(h + 1) * 64, :],
                        zblk[p * 64:(p + 1) * 64, :],
                        xt_t[p * 64:(p + 1) * 64, 0:NL],
                        start=True, stop=True,
                        tile_position=(p * 64, h * 64))

                for e in ch["entries"]:
                    p = e["p"]
                    lhsT = wt_t[p * 64:(p + 1) * 64,
                                (coff + e["loc"]) * BS:(coff + e["loc"] + 1) * BS]
                    rhs = xt_t[p * 64:(p + 1) * 64,
                               e["slot"] * NL:(e["slot"] + 1) * NL]
                    out = ps_tiles[p][e["half"] * 64:(e["half"] + 1) * 64, :]
                    nc.tensor.matmul(out, lhsT, rhs, start=e["start"],
                                     stop=e["stop"],
                                     tile_position=(p * 64, e["half"] * 64))

                b = ob_of[ci]
                if ci == obounds[b]:
                    st_t = st_pool.tile([128, OB * NL], BF16, tag="st",
                                        name=f"st{b}")
                tmp = tmp_pool.tile([128, NL], F32, tag="tmp",
                                    name=f"tmp{ci}")
                soff = (ci - obounds[b]) * NL
                # late-chunk stores go on the sync queue (idle after inbound)
                oeng = nc.scalar if obounds[b] < 20 else nc.sync
                # the last two chunks drain in half-columns for a short
                # ACT->DVE->store latency chain; earlier ones full-width
                parts = ((0, NL // 2), (NL // 2, NL)) \
                    if ci >= NCHUNK - 2 else ((0, NL),)
                for lo, hi in parts:
                    nc.scalar.activation(
                        tmp[:, lo:hi], ps_tiles[0][:, lo:hi],
                        mybir.ActivationFunctionType.Identity,
                        bias=bias_t[:, ci:ci + 1])
                    nc.vector.tensor_tensor(st_t[:, soff + lo:soff + hi],
                                            tmp[:, lo:hi],
                                            ps_tiles[1][:, lo:hi],
                                            op=mybir.AluOpType.add)
                    if ci == NCHUNK - 1:
                        oeng.dma_start(
                            out=yTd[:, ci * NL + lo:ci * NL + hi],
                            in_=st_t[:, soff + lo:soff + hi])
                if ci == obounds[b + 1] - 1 and ci != NCHUNK - 1:
                    n = obounds[b + 1] - obounds[b]
                    oeng.dma_start(
                        out=yTd[:, obounds[b] * NL:obounds[b + 1] * NL],
                        in_=st_t[:, :n * NL])

    _thin_engine_sem_updates(nc, mybir)
    _split_excess_waits(nc, mybir)
    return nc


def kernel(x, blocks, bias, row_idx, col_idx):
    from concourse.bass_utils import run_bass_kernel_spmd

    row_idx = np.asarray(row_idx)
    col_idx = np.asarray(col_idx)
    key = (row_idx.tobytes(), col_idx.tobytes())
    if key not in _CACHE:
        _CACHE[key] = [_plan(row_idx, col_idx), None]
    plan = _CACHE[key][0]

    img, bias_img, seg = _build_images(plan, np.asarray(blocks),
                                       np.asarray(bias, np.float32))
    if _CACHE[key][1] is None:
        _CACHE[key][1] = _build_bass(plan, img.shape[1] // BS, seg,
                                     plan["nslot"])
    nc = _CACHE[key][1]

    # feature row order of the xT image: slot s = (pairs[s][0] block on
    # partitions 0-63, pairs[s][1] on 64-127)
    feat = np.empty((plan["nslot"], 128), np.int64)
    for s, (a, b) in enumerate(plan["pairs"]):
        feat[s, :64] = np.arange(a * BS, (a + 1) * BS)
        feat[s, 64:] = np.arange(b * BS, (b + 1) * BS)

    x = np.asarray(x)
    in_maps = []
    for i in range(NCORES):
        xT = x[i * NL:(i + 1) * NL, :].T.astype(ml_dtypes.bfloat16)
        ximg = np.ascontiguousarray(
            xT[feat.reshape(-1)].reshape(plan["nslot"], 128, NL)
            .swapaxes(0, 1).reshape(128, plan["nslot"] * NL))
        in_maps.append({"xT": ximg, "img": img, "bias_img": bias_img})

    res = run_bass_kernel_spmd(nc, in_maps, list(range(NCORES))).results

    y = np.empty((N_TOK, OUT_F), np.float32)
    for i in range(NCORES):
        raw = np.asarray(res[i]["yT"]).astype(np.float32)
        yl = y[i * NL:(i + 1) * NL]
        for ci, ch in enumerate(plan["chunks"]):
            for (k, h), r in ch["regions"].items():
                yl[:, r * BS:(r + 1) * BS] = \
                    raw[h * 64:(h + 1) * 64, ci * NL:(ci + 1) * NL].T
    return y


# revision 30
# speedup vs baseline: 1.0450x; 1.0450x over previous
"""Block-sparse linear y = x @ W^T + bias on 8 Trainium2 NeuronCores.

W [4096, 4096] has K=1024 dense 64x64 tiles at (row_idx[k], col_idx[k]) on a
64x64 block grid. Data-parallel over tokens: each core gets 512 rows of x and
all blocks, computing yT_local [4096, 512] = W @ x_local^T + bias.

Per-core device layout:
  - xT in SBUF (bf16) as one [128, nslot*512] image: input column blocks
    are paired into "slots" (first member on partitions 0-63, second on
    64-127), so a block's PE quadrant parity is its pair position. The
    pairing and orientation are optimized on the host to balance the four
    (parity, half) quadrant populations per chunk (per-chunk PE time is
    the max quadrant count); slots are ordered by first use across the
    chunk schedule for just-in-time prefetch.
  - all block weights SBUF-resident as one [128, wtot*64] bf16 image of
    B_k^T tiles (parity-0 on partitions 0-63, parity-1 on 64-127).
  - output block-rows are processed in 32 PSUM generations ("chunks") of 2
    rows (one per 64-partition half), rows paired by similar block count;
    parity-p contributions accumulate in psum bank 2k+p (a PSUM
    accumulation group must keep one tile_position). Banks rotate 4-deep.
  - drain: ACT adds bias while copying the even bank to SBUF (half-column
    pieces for a short latency chain), DVE adds the odd bank and writes
    bf16; batched [128, 4*512] bf16 stores go out on the scalar HWDGE
    queue. The host converts back to f32.

All inbound DMAs are EDF-interleaved across the sync and scalar HWDGE
queues (each drains FIFO, sharing HBM at packet granularity) so compute
prerequisites land just in time and the PE, once started, never starves
(the PE p-state ramp 0.65->1.2->2.4 GHz resets on idle, so continuous busy
= full clock; zero-tile warmup matmuls bridge the initial load phase).

The schedule is specialized on the host from row_idx/col_idx (duplicate
(r,c) blocks are pre-summed); all matmul FLOPs run on the PE.

This toolchain's walrus rejects >1 semaphore wait per instruction, so after
Tile scheduling we split excess waits onto same-engine NoOps.
"""

import numpy as np
import ml_dtypes

N_TOK, IN_F, OUT_F, BS, NCORES = 4096, 4096, 4096, 64, 8
NL = N_TOK // NCORES            # tokens per core (512)
GRID = OUT_F // BS              # 64 block-rows / block-cols
NCHUNK = 32                     # psum generations, 2 block-rows each
XB = 4                          # slots per x DMA batch
OB = 4                          # chunks per output DMA batch
NWARM = 24                      # zero-tile warmup matmuls (PE p-state ramp
                                # cover for the initial load phase)
SCAL_CAP = 3 * 1024 * 1024      # inbound bytes allowed on the scalar queue

_CACHE = {}


def _plan(row_idx, col_idx):
    from collections import OrderedDict

    K = int(row_idx.shape[0])
    cells = OrderedDict()
    for k in range(K):
        cells.setdefault((int(row_idx[k]), int(col_idx[k])), []).append(k)
    rows = {r: [] for r in range(GRID)}
    for (r, c) in cells:
        rows[r].append(c)
    counts = {r: len(rows[r]) for r in range(GRID)}

    # chunk 0: the row pair with the smallest column-set union, so the first
    # chunk's x and weight prerequisites are minimal
    best = None
    for r1 in range(GRID):
        for r2 in range(r1 + 1, GRID):
            u = len(set(rows[r1]) | set(rows[r2]))
            if best is None or u < best[0]:
                best = (u, r1, r2)
    first_pair = [best[1], best[2]]

    # remaining chunks pair rows of SIMILAR block count: per-chunk PE time
    # is ceil(max(n_top, n_bot)/2) quadrant-serial matmuls, so similar
    # counts minimize the total
    rest = sorted((r for r in range(GRID) if r not in first_pair),
                  key=lambda r: -counts[r])
    chunk_rows = [[rest[2 * i], rest[2 * i + 1]]
                  for i in range(len(rest) // 2)]

    # order chunks greedily by fewest new columns, compressing the early
    # x-prefetch demand so compute can start while most of x is in flight
    cur = set(c for r in first_pair for c in rows[r])
    rem = list(chunk_rows)
    ordered = [first_pair]
    while rem:
        nxt = min(rem, key=lambda rs: len({c for r in rs for c in rows[r]}
                                          - cur))
        rem.remove(nxt)
        ordered.append(nxt)
        cur |= {c for r in nxt for c in rows[r]}
    # tail chunks big-to-small: the final chunk's matmuls then finish just
    # after the rest, shortening the last drain->store chain
    tail = sorted(ordered[-6:],
                  key=lambda rs: -sum(counts[r] for r in rs))
    chunk_rows = ordered[:-6] + tail

    # half assignment per row (which 64-partition output half of its chunk)
    half_of = {}
    for rs in chunk_rows:
        a, b = sorted(rs)
        half_of[a], half_of[b] = 0, 1
    chunk_of = {r: i for i, rs in enumerate(chunk_rows) for r in rs}
    colrows = {c: [r for r in range(GRID) if c in rows[r]]
               for c in range(GRID)}

    # slot pairing: two columns share a slot (one per partition half, so
    # its parity is pair position). Pair in first-use order for prefetch
    # locality, choosing partners/orientation to balance per-row parity
    # counts (per-chunk PE time is the max quadrant population), then
    # hill-climb orientation flips and windowed partner swaps.
    D = {r: 0 for r in range(GRID)}     # row parity deficit (p0 - p1)

    def orient_score(ca, cb):
        s = 0
        for r in colrows[ca]:
            s += abs(D[r] + 1) - abs(D[r])
        for r in colrows[cb]:
            s += abs(D[r] - 1) - abs(D[r])
        return s

    def commit(ca, cb):
        for r in colrows[ca]:
            D[r] += 1
        for r in colrows[cb]:
            D[r] -= 1

    def best_partner(ca, pend):
        bestp = None
        for j, cb in enumerate(pend):
            for o in (0, 1):
                s = orient_score(ca, cb) if o == 0 else orient_score(cb, ca)
                if bestp is None or s < bestp[0]:
                    bestp = (s, j, o)
        return bestp

    seen, pend, pairs = set(), [], []
    for rs in chunk_rows:
        for c in sorted({c for r in rs for c in rows[r]}):
            if c in seen:
                continue
            seen.add(c)
            pend.append(c)
            if len(pend) >= 6:
                ca = pend.pop(0)
                s, j, o = best_partner(ca, pend)
                cb = pend.pop(j)
                pr = (ca, cb) if o == 0 else (cb, ca)
                pairs.append(list(pr))
                commit(*pr)
    while len(pend) >= 2:
        ca = pend.pop(0)
        s, j, o = best_partner(ca, pend)
        cb = pend.pop(j)
        pr = (ca, cb) if o == 0 else (cb, ca)
        pairs.append(list(pr))
        commit(*pr)
    if pend:
        pairs.append([pend[0], pend[0]])

    def sigma_max(prs):
        par = {}
        for (a, b) in prs:
            par[a] = 0
            if b != a:
                par[b] = 1
        tot = 0
        for rs in chunk_rows:
            q = {(p, h): 0 for p in (0, 1) for h in (0, 1)}
            for r in rs:
                for c in rows[r]:
                    q[(par[c], half_of[r])] += 1
            tot += max(q.values())
        return tot

    cur = sigma_max(pairs)
    for _ in range(40):
        improved = False
        for i in range(len(pairs)):
            pairs[i] = [pairs[i][1], pairs[i][0]]
            s = sigma_max(pairs)
            if s < cur:
                cur, improved = s, True
            else:
                pairs[i] = [pairs[i][1], pairs[i][0]]
        for i in range(len(pairs)):
            for j in range(i + 1, min(i + 7, len(pairs))):
                done = False
                for si, sj in ((0, 0), (0, 1), (1, 0), (1, 1)):
                    a, b = pairs[i][:], pairs[j][:]
                    a[si], b[sj] = b[sj], a[si]
                    old_i, old_j = pairs[i], pairs[j]
                    pairs[i], pairs[j] = a, b
                    s = sigma_max(pairs)
                    if s < cur:
                        cur, improved, done = s, True, True
                        break
                    pairs[i], pairs[j] = old_i, old_j
                if done:
                    break
        if not improved:
            break

    slot, par = {}, {}
    for s, (a, b) in enumerate(pairs):
        slot[a], par[a] = s, 0
        if b != a:
            slot[b], par[b] = s, 1
    nslot = len(pairs)

    chunks = []
    xdead = {}                  # slot -> first chunk index that reads it
    for ci, rs in enumerate(chunk_rows):
        rs = sorted(rs, key=lambda r: -counts[r])
        regions = {(0, 0): rs[0], (0, 1): rs[1]}
        if rs[0] > rs[1]:
            regions = {(0, 0): rs[1], (0, 1): rs[0]}
        rloc = {r: kh for kh, r in regions.items()}

        queues = {(p, h): [] for p in (0, 1) for h in (0, 1)}
        for r in rs:
            k, h = rloc[r]
            for c in sorted(rows[r], key=lambda c: slot[c]):
                queues[(par[c], h)].append((r, c))
                xdead.setdefault(slot[c], ci)

        cycle = [(0, 0), (1, 1), (1, 0), (0, 1)]
        ent = []
        qi = {kq: 0 for kq in queues}
        total = sum(len(q) for q in queues.values())
        while len(ent) < total:
            prog = False
            for kq in cycle:
                q = queues[kq]
                if qi[kq] < len(q):
                    ent.append((q[qi[kq]], kq[0]))
                    qi[kq] += 1
                    prog = True
            assert prog
        # start/stop per accumulator = (region, parity)
        first, last = {}, {}
        for i, ((r, c), p) in enumerate(ent):
            first.setdefault((r, p), i)
            last[(r, p)] = i
        entries = []
        for i, ((r, c), p) in enumerate(ent):
            k, h = rloc[r]
            entries.append(dict(r=r, c=c, p=p, slot=slot[c], half=h,
                               start=(first[(r, p)] == i),
                               stop=(last[(r, p)] == i)))
        # accumulators with no blocks -> memset
        empty = []
        for (k, h), r in regions.items():
            for p in (0, 1):
                if not any(par[c] == p for c in rows[r]):
                    empty.append((k, h, p))
        chunks.append(dict(rows=rs, regions=regions, entries=entries,
                           empty=empty))

    return dict(cells=cells, chunks=chunks, pairs=pairs, nslot=nslot,
                xdead=xdead)


def _build_images(plan, blocks, bias):
    cells = plan["cells"]
    summed = {}
    for key, ks in cells.items():
        if len(ks) == 1:
            summed[key] = blocks[ks[0]]
        else:
            acc = blocks[ks[0]].astype(np.float32).copy()
            for k in ks[1:]:
                acc += blocks[k]
            summed[key] = acc

    # one interleaved image: per chunk, parity-0 blocks on partitions 0-63
    # and parity-1 on 64-127, column-aligned and zero padded to the wider half
    widths = []
    for ch in plan["chunks"]:
        n_e = sum(1 for e in ch["entries"] if e["p"] == 0)
        n_o = len(ch["entries"]) - n_e
        widths.append(max(n_e, n_o, 1))
    wtot = sum(widths)
    img = np.zeros((128, wtot * BS), np.float32)
    seg = []  # per chunk: (col offset in blocks, width in blocks)
    off = 0
    for wch, ch in zip(widths, plan["chunks"]):
        ie = io = 0
        for e in ch["entries"]:
            B = summed[(e["r"], e["c"])]
            if e["p"] == 0:
                img[0:64, (off + ie) * BS:(off + ie + 1) * BS] = B.T
                e["loc"] = ie
                ie += 1
            else:
                img[64:128, (off + io) * BS:(off + io + 1) * BS] = B.T
                e["loc"] = io
                io += 1
        seg.append((off, wch))
        off += wch

    bias_img = np.zeros((128, NCHUNK), np.float32)
    for ci, ch in enumerate(plan["chunks"]):
        for (k, h), r in ch["regions"].items():
            bias_img[h * 64:(h + 1) * 64, ci] = bias[r * BS:(r + 1) * BS]

    return img.astype(ml_dtypes.bfloat16), bias_img, seg


def _split_excess_waits(nc, mybir, limit=1):
    n = 0
    for fn in nc.m.functions:
        for bb in fn.blocks:
            out = []
            for inst in bb.instructions:
                si = inst.sync_info
                if si is not None and si.on_wait and len(si.on_wait) > limit:
                    waits = list(si.on_wait)
                    ups = list(si.on_update)
                    for j, w in enumerate(waits[:-limit]):
                        nop = mybir.InstNoOp(name=f"{inst.name}-ws{j}", ins=[], outs=[])
                        nop.engine = inst.engine
                        nop.sync_info = mybir.SyncInfo(on_wait=[w], on_update=[])
                        out.append(nop)
                        n += 1
                    inst.sync_info = mybir.SyncInfo(on_wait=waits[-limit:], on_update=ups)
                out.append(inst)
            bb.instructions = out
    return n


def _thin_engine_sem_updates(nc, mybir, engines=("EngineType.PE",)):
    """Drop per-instruction +1 sem increments that no wait ever observes.

    Tile gives every engine instruction a `then_inc(engine_sem)`; on the PE a
    serialized EVT_SEM write costs ~26 ns per matmul. Only ticks some other
    instruction actually waits on are needed, so keep increments just before
    each waited tick and renumber all waits by rank.
    """
    insts = []
    for fn in nc.m.functions:
        for bb in fn.blocks:
            insts.extend(bb.instructions)

    from collections import defaultdict
    upd_insts = defaultdict(list)   # sem id -> [instruction, ...] program order
    upd_ok = defaultdict(lambda: True)
    upd_engine = {}
    waited = defaultdict(set)       # sem id -> waited values
    wait_ok = defaultdict(lambda: True)
    for inst in insts:
        si = inst.sync_info
        if si is None:
            continue
        for u in si.on_update:
            if u.sync_type != "semaphore":
                continue
            if u.update_mode != "sem-inc" or u.update_value != 1:
                upd_ok[u.id] = False
            e = str(inst.engine)
            if u.id in upd_engine and upd_engine[u.id] != e:
                upd_ok[u.id] = False
            upd_engine[u.id] = e
            upd_insts[u.id].append(inst)
        for w in si.on_wait:
            if w.sync_type != "semaphore":
                continue
            if w.wait_mode != "sem-ge-imm" or w.wait_reg is not None:
                wait_ok[w.id] = False
            waited[w.id].add(w.wait_value)

    victims = [s for s, il in upd_insts.items()
               if upd_ok[s] and wait_ok[s] and upd_engine.get(s) in engines
               and len(il) > 8]
    for s in victims:
        il = upd_insts[s]
        W = sorted(v for v in waited.get(s, set()) if 1 <= v <= len(il))
        keep_ticks = set(W)
        rank = {v: i + 1 for i, v in enumerate(W)}
        # always keep the final tick so the kernel tail drain can await it
        if len(il) not in keep_ticks:
            keep_ticks.add(len(il))
            rank[len(il)] = len(W) + 1
        for t, inst in enumerate(il, start=1):
            si = inst.sync_info
            ups = [u for u in si.on_update
                   if not (u.sync_type == "semaphore" and u.id == s)]
            if t in keep_ticks:
                ups.append(mybir.SyncUpdate(
                    sync_type="semaphore", id=s, ant_name=f"thin{s}",
                    update_mode="sem-inc", update_value=1, update_reg=None))
            inst.sync_info = mybir.SyncInfo(on_wait=list(si.on_wait),
                                            on_update=ups)
        # renumber waits on this sem everywhere
        for inst in insts:
            si = inst.sync_info
            if si is None or not si.on_wait:
                continue
            changed = False
            ws = []
            for w in si.on_wait:
                if w.sync_type == "semaphore" and w.id == s:
                    nv = rank.get(w.wait_value)
                    if nv is None:
                        nv = sum(1 for v in rank if v <= w.wait_value)
                    ws.append(mybir.SyncWait(
                        sync_type="semaphore", id=s, ant_name=f"thin{s}",
                        wait_mode="sem-ge-imm", wait_value=nv, wait_reg=None))
                    changed = True
                else:
                    ws.append(w)
            if changed:
                inst.sync_info = mybir.SyncInfo(on_wait=ws,
                                                on_update=list(si.on_update))
    return victims


def _build_bass(plan, wimg, seg, nslot):
    import concourse.bass as bass
    import concourse.mybir as mybir
    import concourse.tile as tile

    F32 = mybir.dt.float32
    BF16 = mybir.dt.bfloat16

    nc = bass.Bass()
    xTd = nc.declare_dram_parameter("xT", [128, nslot * NL], BF16, isOutput=False)
    imd = nc.declare_dram_parameter("img", [128, wimg * BS], BF16, isOutput=False)
    bd = nc.declare_dram_parameter("bias_img", [128, NCHUNK], F32, isOutput=False)
    yTd = nc.declare_dram_parameter("yT", [128, NCHUNK * NL], BF16, isOutput=True)

    nxb = (nslot + XB - 1) // XB
    # weight batches: pairs for the first chunks, then fours
    wbounds = [0, 2, 4, 6] + list(range(8, NCHUNK, 4)) + [NCHUNK]
    # output batches: fours, with a small tail for a short drain->store chain
    obounds = list(range(0, NCHUNK - 4, OB)) + [NCHUNK - 4, NCHUNK - 2,
                                                NCHUNK - 1, NCHUNK]
    ob_of = {}
    for b in range(len(obounds) - 1):
        for ci in range(obounds[b], obounds[b + 1]):
            ob_of[ci] = b

    def wslice(b):
        c0, c1 = wbounds[b], wbounds[b + 1] - 1
        return seg[c0][0] * BS, (seg[c1][0] + seg[c1][1]) * BS

    def xslice(b):
        return b * XB * NL, min(nslot, (b + 1) * XB) * NL

    # EDF merge of inbound loads: deadline = first chunk that reads the
    # batch. Weights go on the sync queue (scalar later carries output
    # stores); x batches alternate between the queues up to SCAL_CAP.
    items = []
    for b in range(nxb):
        d = min(plan["xdead"].get(s, NCHUNK)
                for s in range(b * XB, min(nslot, (b + 1) * XB)))
        lo, hi = xslice(b)
        items.append((d, 1, "x", b, (hi - lo) * 256))
    for b in range(len(wbounds) - 1):
        lo, hi = wslice(b)
        items.append((wbounds[b], 0, "w", b, (hi - lo) * 256))
    items.sort()
    # balance the EDF stream across three inbound queues by bytes: weights
    # stay on sync, x also rides scalar (clears before output stores need
    # it) and gpsimd (SWDGE) during the early crunch
    sync_q, scal_q, gp_q = [], [], []
    qb = {"sync": 0.0, "scal": 0.0, "gp": 0.0}
    for d, _, kind, b, sz in items:
        cands = ["sync"]
        if kind == "x":
            if qb["scal"] + sz <= SCAL_CAP:
                cands.append("scal")
            if d <= 10 and qb["gp"] + sz <= 2 * 1024 * 1024:
                cands.append("gp")
        q = min(cands, key=lambda n: qb[n])
        qb[q] += sz
        {"sync": sync_q, "scal": scal_q, "gp": gp_q}[q].append((kind, b))
    scal_q.insert(min(2, len(scal_q)), ("bias", 0))

    with tile.TileContext(nc) as tc:
        with (
            tc.tile_pool(name="big", bufs=1) as big_pool,
            tc.tile_pool(name="cst", bufs=1) as cst_pool,
            tc.tile_pool(name="stp", bufs=3) as st_pool,
            tc.tile_pool(name="tmp", bufs=4) as tmp_pool,
            tc.tile_pool(name="ps", bufs=1, space="PSUM") as ps_pool,
        ):
            zblk = cst_pool.tile([128, BS], BF16, tag="zblk")
            nc.vector.memset(zblk[:], 0.0)
            wscr = cst_pool.tile([128, NL], BF16, tag="wscr")
            nc.vector.memset(wscr[:], 0.0)
            bias_t = cst_pool.tile([128, NCHUNK], F32, tag="bias")

            xt_t = big_pool.tile([128, nslot * NL], BF16, tag="xt")
            wt_t = big_pool.tile([128, wimg * BS], BF16, tag="wt")

            for q, eng in ((sync_q, nc.sync), (scal_q, nc.scalar),
                           (gp_q, nc.gpsimd)):
                for kind, b in q:
                    if kind == "x":
                        lo, hi = xslice(b)
                        eng.dma_start(out=xt_t[:, lo:hi], in_=xTd[:, lo:hi])
                    elif kind == "w":
                        lo, hi = wslice(b)
                        eng.dma_start(out=wt_t[:, lo:hi], in_=imd[:, lo:hi])
                    else:
                        eng.dma_start(out=bias_t[:], in_=bd[:])

            # warmup matmuls on zero tiles: keep the PE busy through the
            # load phase so the p-state/HAM ramp is done when real work lands
            ps_warm = [ps_pool.tile([128, NL], F32, tag=f"bank{6 + p}",
                                    name=f"warm{p}") for p in (0, 1)]
            for i in range(NWARM):
                p, h = (i // 2) % 2, i % 2
                nc.tensor.matmul(ps_warm[p][h * 64:(h + 1) * 64, :],
                                 zblk[p * 64:(p + 1) * 64, :],
                                 wscr[p * 64:(p + 1) * 64, :],
                                 start=True, stop=True,
                                 tile_position=(p * 64, h * 64))

            for ci, ch in enumerate(plan["chunks"]):
                coff = seg[ci][0]
                boff = 2 * (ci % 4)
                ps_tiles = [ps_pool.tile([128, NL], F32, tag=f"bank{boff+b}",
                                         name=f"ps{ci}_{b}")
                            for b in range(2)]
                for (k, h, p) in ch["empty"]:
                    nc.tensor.matmul(
                        ps_tiles[p][h * 64:(h + 1) * 64, :],
                        zblk[p * 64:(p + 1) * 64, :],
                        xt_t[p * 64:(p + 1) * 64, 0:NL],
                        start=True, stop=True,
                        tile_position=(p * 64, h * 64))

                for e in ch["entries"]:
                    p = e["p"]
                    lhsT = wt_t[p * 64:(p + 1) * 64,
                                (coff + e["loc"]) * BS:(coff + e["loc"] + 1) * BS]
                    rhs = xt_t[p * 64:(p + 1) * 64,
                               e["slot"] * NL:(e["slot"] + 1) * NL]
                    out = ps_tiles[p][e["half"] * 64:(e["half"] + 1) * 64, :]
                    nc.tensor.matmul(out, lhsT, rhs, start=e["start"],
                                     stop=e["stop"],
                                     tile_position=(p * 64, e["half"] * 64))

                b = ob_of[ci]
                if ci == obounds[b]:
                    st_t = st_pool.tile([128, OB * NL], BF16, tag="st",
                                        name=f"st{b}")
                tmp = tmp_pool.tile([128, NL], F32, tag="tmp",
                                    name=f"tmp{ci}")
                soff = (ci - obounds[b]) * NL
                # late-chunk stores go on the sync queue (idle after inbound)
                oeng = nc.scalar if obounds[b] < 20 else nc.sync
                # the last two chunks drain in half-columns for a short
                # ACT->DVE->store latency chain; earlier ones full-width
                parts = ((0, NL // 2), (NL // 2, NL)) \
                    if ci >= NCHUNK - 2 else ((0, NL),)
                for lo, hi in parts:
                    nc.scalar.activation(
                        tmp[:, lo:hi], ps_tiles[0][:, lo:hi],
                        mybir.ActivationFunctionType.Identity,
                        bias=bias_t[:, ci:ci + 1])
                    nc.vector.tensor_tensor(st_t[:, soff + lo:soff + hi],
                                            tmp[:, lo:hi],
                                            ps_tiles[1][:, lo:hi],
                                            op=mybir.AluOpType.add)
                    if ci == NCHUNK - 1:
                        oeng.dma_start(
                            out=yTd[:, ci * NL + lo:ci * NL + hi],
                            in_=st_t[:, soff + lo:soff + hi])
                if ci == obounds[b + 1] - 1 and ci != NCHUNK - 1:
                    n = obounds[b + 1] - obounds[b]
                    oeng.dma_start(
                        out=yTd[:, obounds[b] * NL:obounds[b + 1] * NL],
                        in_=st_t[:, :n * NL])

    _thin_engine_sem_updates(nc, mybir)
    _split_excess_waits(nc, mybir)
    return nc


def kernel(x, blocks, bias, row_idx, col_idx):
    from concourse.bass_utils import run_bass_kernel_spmd

    row_idx = np.asarray(row_idx)
    col_idx = np.asarray(col_idx)
    key = (row_idx.tobytes(), col_idx.tobytes())
    if key not in _CACHE:
        _CACHE[key] = [_plan(row_idx, col_idx), None]
    plan = _CACHE[key][0]

    img, bias_img, seg = _build_images(plan, np.asarray(blocks),
                                       np.asarray(bias, np.float32))
    if _CACHE[key][1] is None:
        _CACHE[key][1] = _build_bass(plan, img.shape[1] // BS, seg,
                                     plan["nslot"])
    nc = _CACHE[key][1]

    # feature row order of the xT image: slot s = (pairs[s][0] block on
    # partitions 0-63, pairs[s][1] on 64-127)
    feat = np.empty((plan["nslot"], 128), np.int64)
    for s, (a, b) in enumerate(plan["pairs"]):
        feat[s, :64] = np.arange(a * BS, (a + 1) * BS)
        feat[s, 64:] = np.arange(b * BS, (b + 1) * BS)

    x = np.asarray(x)
    in_maps = []
    for i in range(NCORES):
        xT = x[i * NL:(i + 1) * NL, :].T.astype(ml_dtypes.bfloat16)
        ximg = np.ascontiguousarray(
            xT[feat.reshape(-1)].reshape(plan["nslot"], 128, NL)
            .swapaxes(0, 1).reshape(128, plan["nslot"] * NL))
        in_maps.append({"xT": ximg, "img": img, "bias_img": bias_img})

    res = run_bass_kernel_spmd(nc, in_maps, list(range(NCORES))).results

    y = np.empty((N_TOK, OUT_F), np.float32)
    for i in range(NCORES):
        raw = np.asarray(res[i]["yT"]).astype(np.float32)
        yl = y[i * NL:(i + 1) * NL]
        for ci, ch in enumerate(plan["chunks"]):
            for (k, h), r in ch["regions"].items():
                yl[:, r * BS:(r + 1) * BS] = \
                    raw[h * 64:(h + 1) * 64, ci * NL:(ci + 1) * NL].T
    return y
